# revision 15
# baseline (speedup 1.0000x reference)
"""Distributed 2-layer GCN (DGL GraphConv norm='both') on 8 TRN2 NeuronCores.

Contract: kernel(**inputs) takes the FULL inputs of reference.setup_inputs()
and returns the full (seg_output [1,4], class_activn_map [4,100000]) outputs.

Strategy (one SPMD Bass program, per-core data):
  - Nodes sharded by dst: core c owns nodes [c*12500, (c+1)*12500), i.e. 98
    windows of 128 dst nodes each.
  - Per 128-edge block: indirect gather of src rows (bf16, 256B rows) via
    InstDMAGatherAnt on 4 SWDGE queues; a one-hot scatter matrix
    M[e,d] = (iota==dstloc[e]) built per window with one step-0-broadcast
    tensor_tensor; TensorE matmuls accumulate aggT[f,d] in PSUM.
  - Normalization folding (biases are zero for this problem - asserted - so
    relu commutes with row scalings): norm_src is folded into the features
    on the host; the stored h1 is scaled by norm_src*norm_dst of the node
    (layer-1's nd + layer-2's ns); layer-2's cam rows are scaled by nd.
  - Layer 1 output (transposed back to [node, feat]) is AllGathered
    (3.2MB/rank) into h1_full; layer 2 gathers from it.
  - Layer 2 computes camT[node, cls] = relu(h2T).T @ Wp.T * nd directly from
    the transposed hidden; no h2 store. seg_output = row-mean of cam + bp
    on the host (exactly mean(h2) @ Wp.T + bp).
  - dma_gather indices are int16, so the 100000-row table is covered by 4
    base ranges of 25000 rows. Per core, edges are sorted by
    (batch, range, window, src); each (batch,range) stream is gathered
    compactly (blocks cut at 128 without window alignment, ~4% padding).
    Window w consumes blocks [LO,HI) of each range stream - bounds are
    min/max over cores so one SPMD program fits every core's data - with
    per-(window,block) dstloc columns whose out-of-window lanes are -1
    (zero one-hot column -> no contribution).
"""

import os
import sys

sys.path.insert(0, "/opt/trn_rl_repo")

import numpy as np
import ml_dtypes

from concourse import bass, bacc, mybir, tile
from concourse import bass_utils

bf16_np = ml_dtypes.bfloat16

# ---- problem constants (hardcoded per contract) ----
N = 100000
EDG = 1600000
D = 128
NCLS = 4
NCORES = 8
S = N // NCORES            # 12500 nodes per core
P = 128
NW = (S + P - 1) // P      # 98 windows per core
LASTW = S - (NW - 1) * P   # 84 nodes in last window
NRANGE = 4
RW = 25000                 # range width (<= 32767 for int16 indices)
NBW = 14                   # windows per gather batch
NBATCH = NW // NBW         # 7 batches
assert NW % NBW == 0

f32 = mybir.dt.float32
bf16 = mybir.dt.bfloat16
i16 = mybir.dt.int16

_cache = {}


def _install_profile_hook():
    try:
        import antenv.axon_hooks as axon_hooks
        from trn_agent_boot.trn_boot import _ntff_profile_via_ctypes

        if axon_hooks.get_axon_ntff_profile_hook() is None:
            axon_hooks.set_axon_ntff_profile_hook(
                _ntff_profile_via_ctypes("/opt/axon/libaxon_pjrt.so")
            )
        bass_utils.upload_artifacts = lambda tmpdir: str(tmpdir)
        return True
    except Exception:
        return False


def _pack(core_a, win_a, dstloc_a_f, tbl_idx):
    """Pack one layer's edges: (batch, range, window, src)-sorted compact streams."""
    E = len(core_a)
    bat_a = win_a // NBW
    rng_id = tbl_idx // RW
    rel_idx = (tbl_idx - rng_id * RW).astype(np.int16)

    order = np.lexsort((tbl_idx, win_a, rng_id, bat_a, core_a))
    core_s = core_a[order]
    win_s = win_a[order]
    bat_s = bat_a[order]
    rng_s = rng_id[order]
    rel_s = rel_idx[order]
    dstloc_s = dstloc_a_f[order]

    gs = (core_s * NBATCH + bat_s) * NRANGE + rng_s
    scount = np.bincount(gs, minlength=NCORES * NBATCH * NRANGE).reshape(NCORES, NBATCH, NRANGE)
    SZ = np.ceil(scount / P).astype(np.int64).max(axis=0)
    SZ[:, 0] = np.maximum(SZ[:, 0], 1)
    call_base = np.zeros((NBATCH, NRANGE), np.int64)
    batch_base = np.zeros(NBATCH + 1, np.int64)
    off = 0
    for b in range(NBATCH):
        batch_base[b] = off
        for r in range(NRANGE):
            call_base[b, r] = off
            off += SZ[b, r]
    batch_base[NBATCH] = off
    totblk = int(off)

    _, ginv, gcnt = np.unique(gs, return_inverse=True, return_counts=True)
    first = np.concatenate([[0], np.cumsum(gcnt)[:-1]])
    pos = np.arange(E) - first[ginv]
    blk = pos // P
    lane = (pos % P).astype(np.int64)
    slot = (call_base[bat_s, rng_s] + blk) * P + lane

    cnt_cwr = np.zeros((NCORES, NW, NRANGE), np.int64)
    np.add.at(cnt_cwr, (core_s, win_s, rng_s), 1)
    start_cwr = np.zeros_like(cnt_cwr)
    for b in range(NBATCH):
        ws = slice(b * NBW, (b + 1) * NBW)
        cum = np.cumsum(cnt_cwr[:, ws, :], axis=1)
        start_cwr[:, ws, :] = cum - cnt_cwr[:, ws, :]
    end_cwr = start_cwr + cnt_cwr
    lo_blk = start_cwr // P
    hi_blk = -(-end_cwr // P)
    emptyc = cnt_cwr == 0
    lo_blk = np.where(emptyc, 10 ** 9, lo_blk)
    hi_blk = np.where(emptyc, -1, hi_blk)
    LO = lo_blk.min(axis=0)
    HI = hi_blk.max(axis=0)
    dead = LO > HI.clip(min=0)
    LO = np.where(dead, 0, LO)
    HI = np.where(dead, 0, HI)
    nwb = (HI - LO).clip(min=0)
    fix = nwb.sum(axis=1) == 0
    nwb[fix, 0] = 1
    HI[fix, 0] = LO[fix, 0] + 1

    nblk_w = nwb.sum(axis=1)
    wm_base = np.concatenate([[0], np.cumsum(nblk_w)])
    mcol0 = np.zeros((NW, NRANGE), np.int64)
    for w in range(NW):
        mcol0[w] = wm_base[w] + np.cumsum(np.concatenate([[0], nwb[w, :-1]]))
    totm = int(wm_base[-1])

    totslot = totblk * P
    idx16 = np.zeros((NCORES, P, totslot // 16), np.int16)
    dstloc_arr = np.full((NCORES, P, 2 * totm), -1.0, bf16_np)
    rows = (slot % 16).astype(np.int64)
    cols = slot // 16
    for k in range(8):
        idx16[core_s, rows + 16 * k, cols] = rel_s
    mc = mcol0[win_s, rng_s] + (blk - LO[win_s, rng_s])
    assert (blk >= LO[win_s, rng_s]).all() and (blk < HI[win_s, rng_s]).all()
    dstloc_arr[core_s, lane, 2 * mc] = dstloc_s.astype(bf16_np)
    dstloc_arr[core_s, lane, 2 * mc + 1] = dstloc_s.astype(bf16_np)

    return dict(SZ=SZ, call_base=call_base, batch_base=batch_base, totblk=totblk,
                LO=LO, HI=HI, nwb=nwb, nblk_w=nblk_w, wm_base=wm_base,
                mcol0=mcol0, totm=totm, idx16=idx16, dstloc=dstloc_arr)


# AllGather split: chunk 1 = first NB1 batches of every core's shard
NB1 = 4
CH1 = NB1 * NBW * P        # 7168 nodes per core in chunk 1
CH2 = S - CH1              # 5332 in chunk 2


def _phi_map():
    """Node id -> row in the chunk-concatenated h1_full layout."""
    n = np.arange(N)
    c = n // S
    l = n - c * S
    return np.where(l < CH1, c * CH1 + l, NCORES * CH1 + c * CH2 + (l - CH1))


def _preprocess(src, dst):
    """Host-side index preprocessing: norms, per-layer packed edge data."""
    src = np.asarray(src).astype(np.int64)
    dst = np.asarray(dst).astype(np.int64)

    deg_out = np.bincount(src, minlength=N).astype(np.float32)
    deg_in = np.bincount(dst, minlength=N).astype(np.float32)
    norm_src = np.where(deg_out > 0, 1.0 / np.sqrt(np.maximum(deg_out, 1.0)), 0.0).astype(np.float32)
    norm_dst = np.where(deg_in > 0, 1.0 / np.sqrt(np.maximum(deg_in, 1.0)), 0.0).astype(np.float32)

    core = dst // S
    dloc = dst - core * S
    win = dloc // P
    dstloc = (dloc - win * P).astype(np.float32)

    pack0 = _pack(core, win, dstloc, src)
    pack1 = _pack(core, win, dstloc, _phi_map()[src])

    node = np.arange(NCORES * S)
    sc_nd = norm_dst[:NCORES * S]
    sc_s1 = (norm_src[:NCORES * S] * sc_nd).astype(np.float32)
    s1 = np.zeros((NCORES, P, NW), np.float32)
    s2 = np.zeros((NCORES, P, NW), np.float32)
    cc = node // S
    ll = node % S
    s1[cc, ll % P, ll // P] = sc_s1
    s2[cc, ll % P, ll // P] = sc_nd

    return dict(packs=[pack0, pack1], s1=s1, s2=s2, norm_src=norm_src)


def _bc_iota(iota_ap, nb):
    return bass.AP(iota_ap.tensor, iota_ap.offset,
                   [list(iota_ap.ap[0]), [0, nb], list(iota_ap.ap[1])])


def _bc_inner(ap):
    return bass.AP(ap.tensor, ap.offset,
                   [list(ap.ap[0]), list(ap.ap[1]), [0, P]])


def _build_program(pre, mode="full", nbatch_lim=None):
    packs = pre["packs"]

    nc = bacc.Bacc("TRN2", target_bir_lowering=False, debug=False,
                   num_devices=NCORES, num_swdge_queues=4)

    feat = nc.dram_tensor("feat", [N, D], bf16, kind="ExternalInput")
    idx16_ts = []
    dstloc_ts = []
    for li in range(2):
        pk = packs[li]
        idx16_ts.append(nc.dram_tensor(f"idx16_{li}", [P, pk["totblk"] * 8], i16, kind="ExternalInput"))
        dstloc_ts.append(nc.dram_tensor(f"dstloc_{li}", [P, 2 * pk["totm"]], bf16, kind="ExternalInput"))
    iota_t_d = nc.dram_tensor("iota", [P, P], bf16, kind="ExternalInput")
    ident_d = nc.dram_tensor("ident", [P, P], bf16, kind="ExternalInput")
    w1_d = nc.dram_tensor("w1", [D, D], bf16, kind="ExternalInput")
    w2_d = nc.dram_tensor("w2", [D, D], bf16, kind="ExternalInput")
    wpt_d = nc.dram_tensor("wpt", [D, NCLS], bf16, kind="ExternalInput")
    s1_d = nc.dram_tensor("s1", [P, NW], f32, kind="ExternalInput")
    s2_d = nc.dram_tensor("s2", [P, NW], f32, kind="ExternalInput")
    cam_d = nc.dram_tensor("cam", [P, NW * NCLS], f32, kind="ExternalOutput")

    if mode == "l1":
        h1_sh = nc.dram_tensor("h1_sh", [S, D], bf16, kind="ExternalOutput")
        h1_full = None
    else:
        h1_sh = nc.dram_tensor("h1_sh", [S, D], bf16, kind="Internal")
        h1_full = nc.dram_tensor("h1_full", [N, D], bf16, kind="Internal")

    with tile.TileContext(nc) as tc:
        with (
            tc.tile_pool(name="const", bufs=1) as cpool,
            tc.tile_pool(name="gpool", bufs=2) as gpool,
            tc.tile_pool(name="mpool", bufs=3) as mpool,
            tc.tile_pool(name="epool", bufs=3) as epool,
            tc.tile_pool(name="psA", bufs=2, space="PSUM") as psA,
            tc.tile_pool(name="psB", bufs=2, space="PSUM") as psB,
            tc.tile_pool(name="psC", bufs=2, space="PSUM") as psC,
            tc.tile_pool(name="psD", bufs=2, space="PSUM") as psD,
        ):
            iota_t = cpool.tile([P, P], bf16)
            nc.sync.dma_start(out=iota_t[:], in_=iota_t_d[:])
            ident_t = cpool.tile([P, P], bf16)
            nc.sync.dma_start(out=ident_t[:], in_=ident_d[:])
            w1_t = cpool.tile([D, D], bf16)
            nc.sync.dma_start(out=w1_t[:], in_=w1_d[:])
            w2_t = cpool.tile([D, D], bf16)
            nc.sync.dma_start(out=w2_t[:], in_=w2_d[:])
            wpt_t = cpool.tile([D, NCLS], bf16)
            nc.sync.dma_start(out=wpt_t[:], in_=wpt_d[:])
            s1_t = cpool.tile([P, NW], f32)
            nc.sync.dma_start(out=s1_t[:], in_=s1_d[:])
            s2_t = cpool.tile([P, NW], f32)
            nc.sync.dma_start(out=s2_t[:], in_=s2_d[:])
            cam_stage = cpool.tile([P, NW * NCLS], f32)

            mxgblk = int(max(pk["batch_base"][b + 1] - pk["batch_base"][b]
                             for pk in packs for b in range(NBATCH)))
            mxm_b = int(max(pk["wm_base"][(b + 1) * NBW] - pk["wm_base"][b * NBW]
                            for pk in packs for b in range(NBATCH)))
            mxmblk = int(max(pk["nblk_w"].max() for pk in packs))

            def layer(li, table, b_lo, b_hi):
                pk = packs[li]
                SZ = pk["SZ"]; call_base = pk["call_base"]; batch_base = pk["batch_base"]
                LO = pk["LO"]; nwb = pk["nwb"]; nblk_w = pk["nblk_w"]
                wm_base = pk["wm_base"]; mcol0 = pk["mcol0"]
                idx16_t = idx16_ts[li]; dstloc_t = dstloc_ts[li]
                for b in range(b_lo, b_hi):
                    gb = int(batch_base[b])
                    gnb = int(batch_base[b + 1]) - gb
                    mb = int(wm_base[b * NBW])
                    mnb = int(wm_base[(b + 1) * NBW]) - mb
                    gath = gpool.tile([P, mxgblk, D], bf16, tag="gath")
                    dl_t = gpool.tile([P, 2 * mxm_b], bf16, tag="dl")
                    ix_t = gpool.tile([P, mxgblk * 8], i16, tag="ix")
                    nc.sync.dma_start(out=dl_t[:, :2 * mnb], in_=dstloc_t[:, 2 * mb:2 * (mb + mnb)])
                    nc.sync.dma_start(out=ix_t[:, :gnb * 8], in_=idx16_t[:, gb * 8:(gb + gnb) * 8])

                    for r in range(NRANGE):
                        ncols = int(SZ[b, r])
                        if ncols == 0:
                            continue
                        co = int(call_base[b, r]) - gb
                        hi = min((r + 1) * RW, N)
                        nc.gpsimd.dma_gather(
                            gath[:, co:co + ncols, :],
                            table[r * RW:hi, :],
                            ix_t[:, co * 8:(co + ncols) * 8],
                            ncols * P,
                            ncols * P,
                            D,
                            single_packet=False,
                            queue_num=r,
                        )

                    for wi in range(NBW):
                        w = b * NBW + wi
                        nbw = int(nblk_w[w])
                        mo = int(wm_base[w]) - mb
                        m_t = mpool.tile([P, mxmblk, P], bf16, tag="m")
                        ia = iota_t[:]
                        in0 = bass.AP(ia.tensor, ia.offset,
                                      [list(ia.ap[0]), [0, nbw], [2, P // 2], [1, 2]])
                        pb = dl_t[:, 2 * mo:2 * (mo + nbw)]
                        in1 = bass.AP(pb.tensor, pb.offset,
                                      [list(pb.ap[0]), [2, nbw], [0, P // 2], [1, 2]])
                        nc.vector.tensor_tensor(
                            out=m_t[:, :nbw, :].rearrange("p b (x d) -> p b x d", d=2),
                            in0=in0,
                            in1=in1,
                            op=mybir.AluOpType.is_equal,
                        )
                        aggT_p = psA.tile([P, P], f32, tag="agg")
                        j = 0
                        for r in range(NRANGE):
                            kk = int(nwb[w, r])
                            g0 = int(call_base[b, r]) - gb + int(LO[w, r])
                            m0 = int(mcol0[w, r]) - int(wm_base[w])
                            for k in range(kk):
                                nc.tensor.matmul(
                                    aggT_p[:],
                                    lhsT=gath[:, g0 + k, :],
                                    rhs=m_t[:, m0 + k, :],
                                    start=(j == 0),
                                    stop=(j == nbw - 1),
                                )
                                j += 1
                        aggT_s = epool.tile([P, P], bf16, tag="aggs")
                        nc.vector.tensor_copy(out=aggT_s[:], in_=aggT_p[:])
                        hpre_p = psB.tile([P, P], f32, tag="hpre")
                        wt = w1_t if li == 0 else w2_t
                        nc.tensor.matmul(hpre_p[:], lhsT=wt[:], rhs=aggT_s[:],
                                         start=True, stop=True)
                        hT_s = epool.tile([P, P], bf16, tag="hT")
                        nc.scalar.activation(
                            out=hT_s[:], in_=hpre_p[:],
                            func=mybir.ActivationFunctionType.Relu,
                        )
                        if li == 0:
                            h_p = psC.tile([P, P], bf16, tag="htr")
                            nc.tensor.transpose(out=h_p[:], in_=hT_s[:], identity=ident_t[:])
                            h_s = epool.tile([P, P], bf16, tag="hs")
                            nc.vector.tensor_scalar(
                                out=h_s[:], in0=h_p[:],
                                scalar1=s1_t[:, w:w + 1], scalar2=None,
                                op0=mybir.AluOpType.mult,
                            )
                            wwid = LASTW if w == NW - 1 else P
                            nc.sync.dma_start(
                                out=h1_sh[w * P:w * P + wwid, :], in_=h_s[:wwid, :]
                            )
                        else:
                            cam_p = psD.tile([P, NCLS], f32, tag="cam")
                            nc.tensor.matmul(cam_p[:], lhsT=hT_s[:], rhs=wpt_t[:],
                                             start=True, stop=True)
                            nc.vector.tensor_scalar(
                                out=cam_stage[:, w * NCLS:(w + 1) * NCLS],
                                in0=cam_p[:],
                                scalar1=s2_t[:, w:w + 1], scalar2=None,
                                op0=mybir.AluOpType.mult,
                            )

            nb_full = NBATCH if nbatch_lim is None else nbatch_lim
            layer(0, feat, 0, min(NB1, nb_full))
            if mode != "l1" and nb_full > 0:
                nc.gpsimd.collective_compute(
                    "AllGather",
                    mybir.AluOpType.bypass,
                    replica_groups=[list(range(NCORES))],
                    ins=[h1_sh[0:CH1, :]],
                    outs=[h1_full[0:NCORES * CH1, :]],
                )
            layer(0, feat, min(NB1, nb_full), nb_full)
            if mode != "l1":
                nc.gpsimd.collective_compute(
                    "AllGather",
                    mybir.AluOpType.bypass,
                    replica_groups=[list(range(NCORES))],
                    ins=[h1_sh[CH1:S, :]],
                    outs=[h1_full[NCORES * CH1:N, :]],
                )
            if mode == "full":
                layer(1, h1_full, 0, nb_full)
            else:
                nc.vector.memset(cam_stage[:], 0.0)
            nc.sync.dma_start(out=cam_d[:], in_=cam_stage[:])

    nc.compile()
    return nc


def _make_in_maps(pre, features, W1, W2, Wp):
    feat_ns = np.asarray(features, np.float32) * pre["norm_src"][:, None]
    in_common = {
        "feat": feat_ns.astype(bf16_np),
        "iota": np.broadcast_to(np.arange(P), (P, P)).astype(bf16_np),
        "ident": np.eye(P).astype(bf16_np),
        "w1": np.asarray(W1, np.float32).astype(bf16_np),
        "w2": np.asarray(W2, np.float32).astype(bf16_np),
        "wpt": np.ascontiguousarray(np.asarray(Wp, np.float32).T).astype(bf16_np),
    }
    in_maps = []
    for c in range(NCORES):
        m = dict(in_common)
        for li in range(2):
            m[f"idx16_{li}"] = pre["packs"][li]["idx16"][c]
            m[f"dstloc_{li}"] = pre["packs"][li]["dstloc"][c]
        m["s1"] = pre["s1"][c]
        m["s2"] = pre["s2"][c]
        in_maps.append(m)
    return in_maps


def kernel(features, src, dst, is_training, W1, b1, W2, b2, Wp, bp):
    b1 = np.asarray(b1, np.float32)
    b2 = np.asarray(b2, np.float32)
    assert np.all(b1 == 0) and np.all(b2 == 0), (
        "kernel specialization assumes zero hidden biases (true for this problem)"
    )
    key = (hash(np.asarray(src).tobytes()) ^ hash(np.asarray(dst).tobytes()))
    if key not in _cache:
        pre = _preprocess(src, dst)
        nc = _build_program(pre)
        _cache[key] = (pre, nc)
    pre, nc = _cache[key]

    in_maps = _make_in_maps(pre, features, W1, W2, Wp)

    trace = os.environ.get("GCN_TRACE", "0") == "1"
    if trace:
        _install_profile_hook()
    res = bass_utils.run_bass_kernel_spmd(
        nc, in_maps, core_ids=list(range(NCORES)), trace=trace
    )
    if trace and res.exec_time_ns is not None:
        print(f"HW exec time: {res.exec_time_ns} ns")

    bp = np.asarray(bp, np.float32)
    cam_parts = []
    for c in range(NCORES):
        raw = res.results[c]["cam"].reshape(P, NW, NCLS)
        camT = raw.transpose(1, 0, 2).reshape(NW * P, NCLS)[:S]  # [node, cls]
        cam_parts.append(camT.T)
    cam = np.concatenate(cam_parts, axis=1).astype(np.float32)
    hg = cam.astype(np.float64).sum(axis=1) / N
    seg = (hg + bp.astype(np.float64)).astype(np.float32).reshape(1, NCLS)
    return seg, cam


# revision 16
# speedup vs baseline: 1.3064x; 1.3064x over previous
"""Distributed 2-layer GCN (DGL GraphConv norm='both') on 8 TRN2 NeuronCores.

Contract: kernel(**inputs) takes the FULL inputs of reference.setup_inputs()
and returns the full (seg_output [1,4], class_activn_map [4,100000]) outputs.

Strategy (one SPMD Bass program, per-core data):
  - Nodes sharded by dst: core c owns nodes [c*12500, (c+1)*12500), i.e. 98
    windows of 128 dst nodes each.
  - Per 128-edge block: indirect gather of src rows (bf16, 256B rows) via
    InstDMAGatherAnt on 4 SWDGE queues; a one-hot scatter matrix
    M[e,d] = (iota==dstloc[e]) built per window with one step-0-broadcast
    tensor_tensor; TensorE matmuls accumulate aggT[f,d] in PSUM.
  - Normalization folding (biases are zero for this problem - asserted - so
    relu commutes with row scalings): norm_src is folded into the features
    on the host; the stored h1 is scaled by norm_src*norm_dst of the node
    (layer-1's nd + layer-2's ns); layer-2's cam rows are scaled by nd.
  - Layer 1 output (transposed back to [node, feat]) is AllGathered
    (3.2MB/rank) into h1_full; layer 2 gathers from it.
  - Layer 2 computes camT[node, cls] = relu(h2T).T @ Wp.T * nd directly from
    the transposed hidden; no h2 store. seg_output = row-mean of cam + bp
    on the host (exactly mean(h2) @ Wp.T + bp).
  - dma_gather indices are int16, so the 100000-row table is covered by 4
    base ranges of 25000 rows. Per core, edges are sorted by
    (batch, range, window, src); each (batch,range) stream is gathered
    compactly (blocks cut at 128 without window alignment, ~4% padding).
    Window w consumes blocks [LO,HI) of each range stream - bounds are
    min/max over cores so one SPMD program fits every core's data - with
    per-(window,block) dstloc columns whose out-of-window lanes are -1
    (zero one-hot column -> no contribution).
"""

import os
import sys

sys.path.insert(0, "/opt/trn_rl_repo")

import numpy as np
import ml_dtypes

from concourse import bass, bacc, mybir, tile
from concourse import bass_utils

bf16_np = ml_dtypes.bfloat16

# ---- problem constants (hardcoded per contract) ----
N = 100000
EDG = 1600000
D = 128
NCLS = 4
NCORES = 8
S = N // NCORES            # 12500 nodes per core
P = 128
NW = (S + P - 1) // P      # 98 windows per core
LASTW = S - (NW - 1) * P   # 84 nodes in last window
NRANGE = 4
RW = 25000                 # range width (<= 32767 for int16 indices)
NBW = 7                    # windows per gather batch
NBATCH = NW // NBW         # 14 batches
assert NW % NBW == 0

f32 = mybir.dt.float32
bf16 = mybir.dt.bfloat16
i16 = mybir.dt.int16

_cache = {}


def _install_profile_hook():
    try:
        import antenv.axon_hooks as axon_hooks
        from trn_agent_boot.trn_boot import _ntff_profile_via_ctypes

        if axon_hooks.get_axon_ntff_profile_hook() is None:
            axon_hooks.set_axon_ntff_profile_hook(
                _ntff_profile_via_ctypes("/opt/axon/libaxon_pjrt.so")
            )
        bass_utils.upload_artifacts = lambda tmpdir: str(tmpdir)
        return True
    except Exception:
        return False


def _pack(core_a, win_a, dstloc_a_f, tbl_idx):
    """Pack one layer's edges: (batch, range, window, src)-sorted compact streams."""
    E = len(core_a)
    bat_a = win_a // NBW
    rng_id = tbl_idx // RW
    rel_idx = (tbl_idx - rng_id * RW).astype(np.int16)

    order = np.lexsort((tbl_idx, win_a, rng_id, bat_a, core_a))
    core_s = core_a[order]
    win_s = win_a[order]
    bat_s = bat_a[order]
    rng_s = rng_id[order]
    rel_s = rel_idx[order]
    dstloc_s = dstloc_a_f[order]

    gs = (core_s * NBATCH + bat_s) * NRANGE + rng_s
    scount = np.bincount(gs, minlength=NCORES * NBATCH * NRANGE).reshape(NCORES, NBATCH, NRANGE)
    SZ = np.ceil(scount / P).astype(np.int64).max(axis=0)
    SZ[:, 0] = np.maximum(SZ[:, 0], 1)
    call_base = np.zeros((NBATCH, NRANGE), np.int64)
    batch_base = np.zeros(NBATCH + 1, np.int64)
    off = 0
    for b in range(NBATCH):
        batch_base[b] = off
        for r in range(NRANGE):
            call_base[b, r] = off
            off += SZ[b, r]
    batch_base[NBATCH] = off
    totblk = int(off)

    _, ginv, gcnt = np.unique(gs, return_inverse=True, return_counts=True)
    first = np.concatenate([[0], np.cumsum(gcnt)[:-1]])
    pos = np.arange(E) - first[ginv]
    blk = pos // P
    lane = (pos % P).astype(np.int64)
    slot = (call_base[bat_s, rng_s] + blk) * P + lane

    cnt_cwr = np.zeros((NCORES, NW, NRANGE), np.int64)
    np.add.at(cnt_cwr, (core_s, win_s, rng_s), 1)
    start_cwr = np.zeros_like(cnt_cwr)
    for b in range(NBATCH):
        ws = slice(b * NBW, (b + 1) * NBW)
        cum = np.cumsum(cnt_cwr[:, ws, :], axis=1)
        start_cwr[:, ws, :] = cum - cnt_cwr[:, ws, :]
    end_cwr = start_cwr + cnt_cwr
    lo_blk = start_cwr // P
    hi_blk = -(-end_cwr // P)
    emptyc = cnt_cwr == 0
    lo_blk = np.where(emptyc, 10 ** 9, lo_blk)
    hi_blk = np.where(emptyc, -1, hi_blk)
    LO = lo_blk.min(axis=0)
    HI = hi_blk.max(axis=0)
    dead = LO > HI.clip(min=0)
    LO = np.where(dead, 0, LO)
    HI = np.where(dead, 0, HI)
    nwb = (HI - LO).clip(min=0)
    fix = nwb.sum(axis=1) == 0
    nwb[fix, 0] = 1
    HI[fix, 0] = LO[fix, 0] + 1

    nblk_w = nwb.sum(axis=1)
    wm_base = np.concatenate([[0], np.cumsum(nblk_w)])
    mcol0 = np.zeros((NW, NRANGE), np.int64)
    for w in range(NW):
        mcol0[w] = wm_base[w] + np.cumsum(np.concatenate([[0], nwb[w, :-1]]))
    totm = int(wm_base[-1])

    totslot = totblk * P
    idx16 = np.zeros((NCORES, P, totslot // 16), np.int16)
    dstloc_arr = np.full((NCORES, P, 2 * totm), -1.0, bf16_np)
    rows = (slot % 16).astype(np.int64)
    cols = slot // 16
    for k in range(8):
        idx16[core_s, rows + 16 * k, cols] = rel_s
    mc = mcol0[win_s, rng_s] + (blk - LO[win_s, rng_s])
    assert (blk >= LO[win_s, rng_s]).all() and (blk < HI[win_s, rng_s]).all()
    dstloc_arr[core_s, lane, 2 * mc] = dstloc_s.astype(bf16_np)
    dstloc_arr[core_s, lane, 2 * mc + 1] = dstloc_s.astype(bf16_np)

    return dict(SZ=SZ, call_base=call_base, batch_base=batch_base, totblk=totblk,
                LO=LO, HI=HI, nwb=nwb, nblk_w=nblk_w, wm_base=wm_base,
                mcol0=mcol0, totm=totm, idx16=idx16, dstloc=dstloc_arr)


# AllGather split: chunk 1 = first NB1 batches of every core's shard
NB1 = 8
CH1 = NB1 * NBW * P        # 7168 nodes per core in chunk 1
CH2 = S - CH1              # 5332 in chunk 2


def _phi_map():
    """Node id -> row in the chunk-concatenated h1_full layout."""
    n = np.arange(N)
    c = n // S
    l = n - c * S
    return np.where(l < CH1, c * CH1 + l, NCORES * CH1 + c * CH2 + (l - CH1))


def _preprocess(src, dst):
    """Host-side index preprocessing: norms, per-layer packed edge data."""
    src = np.asarray(src).astype(np.int64)
    dst = np.asarray(dst).astype(np.int64)

    deg_out = np.bincount(src, minlength=N).astype(np.float32)
    deg_in = np.bincount(dst, minlength=N).astype(np.float32)
    norm_src = np.where(deg_out > 0, 1.0 / np.sqrt(np.maximum(deg_out, 1.0)), 0.0).astype(np.float32)
    norm_dst = np.where(deg_in > 0, 1.0 / np.sqrt(np.maximum(deg_in, 1.0)), 0.0).astype(np.float32)

    core = dst // S
    dloc = dst - core * S
    win = dloc // P
    dstloc = (dloc - win * P).astype(np.float32)

    pack0 = _pack(core, win, dstloc, src)
    pack1 = _pack(core, win, dstloc, _phi_map()[src])

    node = np.arange(NCORES * S)
    sc_nd = norm_dst[:NCORES * S]
    sc_s1 = (norm_src[:NCORES * S] * sc_nd).astype(np.float32)
    s1 = np.zeros((NCORES, P, NW), np.float32)
    s2 = np.zeros((NCORES, P, NW), np.float32)
    cc = node // S
    ll = node % S
    s1[cc, ll % P, ll // P] = sc_s1
    s2[cc, ll % P, ll // P] = sc_nd

    return dict(packs=[pack0, pack1], s1=s1, s2=s2, norm_src=norm_src)


def _bc_iota(iota_ap, nb):
    return bass.AP(iota_ap.tensor, iota_ap.offset,
                   [list(iota_ap.ap[0]), [0, nb], list(iota_ap.ap[1])])


def _bc_inner(ap):
    return bass.AP(ap.tensor, ap.offset,
                   [list(ap.ap[0]), list(ap.ap[1]), [0, P]])


def _build_program(pre, mode="full", nbatch_lim=None):
    packs = pre["packs"]

    nc = bacc.Bacc("TRN2", target_bir_lowering=False, debug=False,
                   num_devices=NCORES, num_swdge_queues=4)

    feat = nc.dram_tensor("feat", [N, D], bf16, kind="ExternalInput")
    idx16_ts = []
    dstloc_ts = []
    for li in range(2):
        pk = packs[li]
        idx16_ts.append(nc.dram_tensor(f"idx16_{li}", [P, pk["totblk"] * 8], i16, kind="ExternalInput"))
        dstloc_ts.append(nc.dram_tensor(f"dstloc_{li}", [P, 2 * pk["totm"]], bf16, kind="ExternalInput"))
    iota_t_d = nc.dram_tensor("iota", [P, P], bf16, kind="ExternalInput")
    ident_d = nc.dram_tensor("ident", [P, P], bf16, kind="ExternalInput")
    w1_d = nc.dram_tensor("w1", [D, D], bf16, kind="ExternalInput")
    w2_d = nc.dram_tensor("w2", [D, D], bf16, kind="ExternalInput")
    wpt_d = nc.dram_tensor("wpt", [D, NCLS], bf16, kind="ExternalInput")
    s1_d = nc.dram_tensor("s1", [P, NW], f32, kind="ExternalInput")
    s2_d = nc.dram_tensor("s2", [P, NW], f32, kind="ExternalInput")
    cam_d = nc.dram_tensor("cam", [P, NW * NCLS], f32, kind="ExternalOutput")

    if mode == "l1":
        h1_sh = nc.dram_tensor("h1_sh", [S, D], bf16, kind="ExternalOutput")
        h1_full = None
    else:
        h1_sh = nc.dram_tensor("h1_sh", [S, D], bf16, kind="Internal")
        h1_full = nc.dram_tensor("h1_full", [N, D], bf16, kind="Internal", addr_space="Shared")

    with tile.TileContext(nc) as tc:
        with (
            tc.tile_pool(name="const", bufs=1) as cpool,
            tc.tile_pool(name="gpool", bufs=3) as gpool,
            tc.tile_pool(name="mpool", bufs=3) as mpool,
            tc.tile_pool(name="epool", bufs=3) as epool,
            tc.tile_pool(name="psA", bufs=2, space="PSUM") as psA,
            tc.tile_pool(name="psB", bufs=2, space="PSUM") as psB,
            tc.tile_pool(name="psC", bufs=2, space="PSUM") as psC,
            tc.tile_pool(name="psD", bufs=2, space="PSUM") as psD,
        ):
            iota_t = cpool.tile([P, P], bf16)
            nc.sync.dma_start(out=iota_t[:], in_=iota_t_d[:])
            ident_t = cpool.tile([P, P], bf16)
            nc.sync.dma_start(out=ident_t[:], in_=ident_d[:])
            w1_t = cpool.tile([D, D], bf16)
            nc.sync.dma_start(out=w1_t[:], in_=w1_d[:])
            w2_t = cpool.tile([D, D], bf16)
            nc.sync.dma_start(out=w2_t[:], in_=w2_d[:])
            wpt_t = cpool.tile([D, NCLS], bf16)
            nc.sync.dma_start(out=wpt_t[:], in_=wpt_d[:])
            s1_t = cpool.tile([P, NW], f32)
            nc.sync.dma_start(out=s1_t[:], in_=s1_d[:])
            s2_t = cpool.tile([P, NW], f32)
            nc.sync.dma_start(out=s2_t[:], in_=s2_d[:])
            cam_stage = cpool.tile([P, NW * NCLS], f32)

            mxgblk = int(max(pk["batch_base"][b + 1] - pk["batch_base"][b]
                             for pk in packs for b in range(NBATCH)))
            mxm_b = int(max(pk["wm_base"][(b + 1) * NBW] - pk["wm_base"][b * NBW]
                            for pk in packs for b in range(NBATCH)))
            mxmblk = int(max(pk["nblk_w"].max() for pk in packs))

            def layer(li, table, b_lo, b_hi):
                pk = packs[li]
                SZ = pk["SZ"]; call_base = pk["call_base"]; batch_base = pk["batch_base"]
                LO = pk["LO"]; nwb = pk["nwb"]; nblk_w = pk["nblk_w"]
                wm_base = pk["wm_base"]; mcol0 = pk["mcol0"]
                idx16_t = idx16_ts[li]; dstloc_t = dstloc_ts[li]
                for b in range(b_lo, b_hi):
                    gb = int(batch_base[b])
                    gnb = int(batch_base[b + 1]) - gb
                    mb = int(wm_base[b * NBW])
                    mnb = int(wm_base[(b + 1) * NBW]) - mb
                    gath = gpool.tile([P, mxgblk, D], bf16, tag="gath")
                    dl_t = gpool.tile([P, 2 * mxm_b], bf16, tag="dl")
                    ix_t = gpool.tile([P, mxgblk * 8], i16, tag="ix")
                    nc.sync.dma_start(out=dl_t[:, :2 * mnb], in_=dstloc_t[:, 2 * mb:2 * (mb + mnb)])
                    nc.sync.dma_start(out=ix_t[:, :gnb * 8], in_=idx16_t[:, gb * 8:(gb + gnb) * 8])

                    for r in range(NRANGE):
                        ncols = int(SZ[b, r])
                        if ncols == 0:
                            continue
                        co = int(call_base[b, r]) - gb
                        hi = min((r + 1) * RW, N)
                        nc.gpsimd.dma_gather(
                            gath[:, co:co + ncols, :],
                            table[r * RW:hi, :],
                            ix_t[:, co * 8:(co + ncols) * 8],
                            ncols * P,
                            ncols * P,
                            D,
                            single_packet=False,
                            queue_num=r,
                        )

                    for wi in range(NBW):
                        w = b * NBW + wi
                        nbw = int(nblk_w[w])
                        mo = int(wm_base[w]) - mb
                        m_t = mpool.tile([P, mxmblk, P], bf16, tag="m")
                        ia = iota_t[:]
                        in0 = bass.AP(ia.tensor, ia.offset,
                                      [list(ia.ap[0]), [0, nbw], [2, P // 2], [1, 2]])
                        pb = dl_t[:, 2 * mo:2 * (mo + nbw)]
                        in1 = bass.AP(pb.tensor, pb.offset,
                                      [list(pb.ap[0]), [2, nbw], [0, P // 2], [1, 2]])
                        nc.vector.tensor_tensor(
                            out=m_t[:, :nbw, :].rearrange("p b (x d) -> p b x d", d=2),
                            in0=in0,
                            in1=in1,
                            op=mybir.AluOpType.is_equal,
                        )
                        aggT_p = psA.tile([P, P], f32, tag="agg")
                        j = 0
                        for r in range(NRANGE):
                            kk = int(nwb[w, r])
                            g0 = int(call_base[b, r]) - gb + int(LO[w, r])
                            m0 = int(mcol0[w, r]) - int(wm_base[w])
                            for k in range(kk):
                                nc.tensor.matmul(
                                    aggT_p[:],
                                    lhsT=gath[:, g0 + k, :],
                                    rhs=m_t[:, m0 + k, :],
                                    start=(j == 0),
                                    stop=(j == nbw - 1),
                                )
                                j += 1
                        aggT_s = epool.tile([P, P], bf16, tag="aggs")
                        nc.vector.tensor_copy(out=aggT_s[:], in_=aggT_p[:])
                        hpre_p = psB.tile([P, P], f32, tag="hpre")
                        wt = w1_t if li == 0 else w2_t
                        nc.tensor.matmul(hpre_p[:], lhsT=wt[:], rhs=aggT_s[:],
                                         start=True, stop=True)
                        hT_s = epool.tile([P, P], bf16, tag="hT")
                        nc.scalar.activation(
                            out=hT_s[:], in_=hpre_p[:],
                            func=mybir.ActivationFunctionType.Relu,
                        )
                        if li == 0:
                            h_p = psC.tile([P, P], bf16, tag="htr")
                            nc.tensor.transpose(out=h_p[:], in_=hT_s[:], identity=ident_t[:])
                            h_s = epool.tile([P, P], bf16, tag="hs")
                            nc.vector.tensor_scalar(
                                out=h_s[:], in0=h_p[:],
                                scalar1=s1_t[:, w:w + 1], scalar2=None,
                                op0=mybir.AluOpType.mult,
                            )
                            wwid = LASTW if w == NW - 1 else P
                            nc.sync.dma_start(
                                out=h1_sh[w * P:w * P + wwid, :], in_=h_s[:wwid, :]
                            )
                        else:
                            cam_p = psD.tile([P, NCLS], f32, tag="cam")
                            nc.tensor.matmul(cam_p[:], lhsT=hT_s[:], rhs=wpt_t[:],
                                             start=True, stop=True)
                            nc.vector.tensor_scalar(
                                out=cam_stage[:, w * NCLS:(w + 1) * NCLS],
                                in0=cam_p[:],
                                scalar1=s2_t[:, w:w + 1], scalar2=None,
                                op0=mybir.AluOpType.mult,
                            )

            nb_full = NBATCH if nbatch_lim is None else nbatch_lim
            layer(0, feat, 0, min(NB1, nb_full))
            if mode != "l1" and nb_full > 0:
                nc.gpsimd.collective_compute(
                    "AllGather",
                    mybir.AluOpType.bypass,
                    replica_groups=[list(range(NCORES))],
                    ins=[h1_sh[0:CH1, :]],
                    outs=[h1_full[0:NCORES * CH1, :]],
                )
            layer(0, feat, min(NB1, nb_full), nb_full)
            if mode != "l1":
                nc.gpsimd.collective_compute(
                    "AllGather",
                    mybir.AluOpType.bypass,
                    replica_groups=[list(range(NCORES))],
                    ins=[h1_sh[CH1:S, :]],
                    outs=[h1_full[NCORES * CH1:N, :]],
                )
            if mode == "full":
                layer(1, h1_full, 0, nb_full)
            else:
                nc.vector.memset(cam_stage[:], 0.0)
            nc.sync.dma_start(out=cam_d[:], in_=cam_stage[:])

    nc.compile()
    return nc


def _make_in_maps(pre, features, W1, W2, Wp):
    feat_ns = np.asarray(features, np.float32) * pre["norm_src"][:, None]
    in_common = {
        "feat": feat_ns.astype(bf16_np),
        "iota": np.broadcast_to(np.arange(P), (P, P)).astype(bf16_np),
        "ident": np.eye(P).astype(bf16_np),
        "w1": np.asarray(W1, np.float32).astype(bf16_np),
        "w2": np.asarray(W2, np.float32).astype(bf16_np),
        "wpt": np.ascontiguousarray(np.asarray(Wp, np.float32).T).astype(bf16_np),
    }
    in_maps = []
    for c in range(NCORES):
        m = dict(in_common)
        for li in range(2):
            m[f"idx16_{li}"] = pre["packs"][li]["idx16"][c]
            m[f"dstloc_{li}"] = pre["packs"][li]["dstloc"][c]
        m["s1"] = pre["s1"][c]
        m["s2"] = pre["s2"][c]
        in_maps.append(m)
    return in_maps


def kernel(features, src, dst, is_training, W1, b1, W2, b2, Wp, bp):
    b1 = np.asarray(b1, np.float32)
    b2 = np.asarray(b2, np.float32)
    assert np.all(b1 == 0) and np.all(b2 == 0), (
        "kernel specialization assumes zero hidden biases (true for this problem)"
    )
    key = (hash(np.asarray(src).tobytes()) ^ hash(np.asarray(dst).tobytes()))
    if key not in _cache:
        pre = _preprocess(src, dst)
        nc = _build_program(pre)
        _cache[key] = (pre, nc)
    pre, nc = _cache[key]

    in_maps = _make_in_maps(pre, features, W1, W2, Wp)

    trace = os.environ.get("GCN_TRACE", "0") == "1"
    if trace:
        _install_profile_hook()
    res = bass_utils.run_bass_kernel_spmd(
        nc, in_maps, core_ids=list(range(NCORES)), trace=trace
    )
    if trace and res.exec_time_ns is not None:
        print(f"HW exec time: {res.exec_time_ns} ns")

    bp = np.asarray(bp, np.float32)
    cam_parts = []
    for c in range(NCORES):
        raw = res.results[c]["cam"].reshape(P, NW, NCLS)
        camT = raw.transpose(1, 0, 2).reshape(NW * P, NCLS)[:S]  # [node, cls]
        cam_parts.append(camT.T)
    cam = np.concatenate(cam_parts, axis=1).astype(np.float32)
    hg = cam.astype(np.float64).sum(axis=1) / N
    seg = (hg + bp.astype(np.float64)).astype(np.float32).reshape(1, NCLS)
    return seg, cam
